# revision 7
# baseline (speedup 1.0000x reference)
"""Bass/Trainium2 SPMD kernel for the bi-branch FNO (nn_BiFNOk).

Sharding (8 cores):
  - Spatial rows: core k owns rows [32k,32k+32) of the 256-grid branch (A)
    and rows [16k,16k+16) of the 128-grid branch (B).
  - Spectral modes: core k owns row-frequency block f in [4k,4k+4) (all 32
    column modes); the big weight A is sharded accordingly so each core
    streams 1/8 of A per pass.
  - Per layer: forward DFT partials (local rows) -> ReduceScatter (sum over
    cores, scatter f-blocks) -> per-mode complex 96x96 GEMM -> AllGather ->
    inverse DFT + conv MLP on local rows (3x3-conv halo handled by
    evaluating the inverse DFT on extra rows; zeros at the global boundary
    are baked into the per-core inverse row-DFT matrix).

Heavy matmuls run in bf16 with fp32 PSUM accumulation; the residual state
stays fp32. The truncated (32x32-mode) FFTs are exact small DFT matmuls.
"""

import numpy as np
import ml_dtypes

import concourse.bacc as bacc
import concourse.mybir as mybir
from concourse.bass_utils import run_bass_kernel_spmd
from concourse.tile import TileContext

F32 = mybir.dt.float32
BF16 = mybir.dt.bfloat16

L = 4
M = 32          # modes per dim
P = 96
HA, WA = 256, 256
HB, WB = 128, 128
NC = 8
RA = HA // NC   # 32 rows of branch A per core
RB = HB // NC   # 16 rows of branch B per core
RBH = RB + 4    # B rows incl. halo of 2 each side -> 20
FL = M // NC    # 4 f-rows per core
GELU = mybir.ActivationFunctionType.Gelu_apprx_tanh
COPYF = mybir.ActivationFunctionType.Copy

_CACHE = {}
DBG = False
NL = L


def _bf(x):
    return np.ascontiguousarray(np.asarray(x)).astype(ml_dtypes.bfloat16)


def _f32(x):
    return np.ascontiguousarray(np.asarray(x)).astype(np.float32)


def _dft_consts(core):
    """Per-core DFT matrices (bf16)."""
    out = {}
    n = np.arange(M)
    f = np.arange(M) - M // 2
    for tag, H in (("a", HA), ("b", HB)):
        w = np.arange(H)
        # fwd stage1: lhsT Ec [w, (ri_c2, n32)]
        Ec = np.zeros((H, 64), np.float32)
        Ec[:, :M] = np.cos(2 * np.pi * np.outer(w, n) / H) / H
        Ec[:, M:] = -np.sin(2 * np.pi * np.outer(w, n) / H) / H
        out[f"ec_{tag}"] = _bf(Ec)
        # fwd stage2: lhsT ErC [(ri_c2, h32), (ri2, f32)], local rows
        nrow = RA if tag == "a" else RB
        hrows = core * nrow + np.arange(nrow)
        ErC = np.zeros((64, 64), np.float32)
        th = 2 * np.pi * np.outer(hrows, f) / H       # [h, f]
        c, s = np.cos(th) / H, np.sin(th) / H
        ErC[0:nrow, 0:M] = c            # z_re -> coeff_re
        ErC[32:32 + nrow, 0:M] = s      # z_im -> coeff_re
        ErC[0:nrow, M:] = -s            # z_re -> coeff_im
        ErC[32:32 + nrow, M:] = c       # z_im -> coeff_im
        out[f"erc_{tag}"] = _bf(ErC)
        # inv stage1: lhsT GrC [(ri'2, f32), (ri_t2, h32)]
        if tag == "a":
            hg = core * RA + np.arange(RA)
            valid = np.ones(RA, bool)
        else:
            hg = core * RB - 2 + np.arange(RBH)
            valid = (hg >= 0) & (hg < HB)
        GrC = np.zeros((64, 64), np.float32)
        th = 2 * np.pi * np.outer(f, hg) / H          # [f, h]
        c, s = np.cos(th), np.sin(th)
        c[:, ~valid] = 0.0
        s[:, ~valid] = 0.0
        R_ = len(hg)
        GrC[0:M, 0:R_] = c              # c_re -> t_re
        GrC[M:, 0:R_] = -s              # c_im -> t_re
        GrC[0:M, 32:32 + R_] = s        # c_re -> t_im
        GrC[M:, 32:32 + R_] = c         # c_im -> t_im
        out[f"grc_{tag}"] = _bf(GrC)
        # inv stage2 (moving operand): Gc [(ri_t2, n32), w]
        al = np.where(n == 0, 1.0, 2.0)
        Gc = np.zeros((64, H), np.float32)
        Gc[0:M] = al[:, None] * np.cos(2 * np.pi * np.outer(n, w) / H)
        Gc[M:] = -al[:, None] * np.sin(2 * np.pi * np.outer(n, w) / H)
        out[f"gc_{tag}"] = _bf(Gc)
    return out


def _build():
    nc = bacc.Bacc(None)
    dp = nc.declare_dram_parameter

    ina = dp("ina", [3, RA, WA], F32, isOutput=False)
    inb = dp("inb", [3, RB, WB], F32, isOutput=False)
    enc_a_w = dp("enc_a_w", [3, 48], BF16, isOutput=False)
    enc_b_w = dp("enc_b_w", [3, 48], BF16, isOutput=False)
    enc_a_b = dp("enc_a_b", [48, 1], F32, isOutput=False)
    enc_b_b = dp("enc_b_b", [48, 1], F32, isOutput=False)
    dec_a_w = dp("dec_a_w", [48, 1], F32, isOutput=False)
    dec_b_w = dp("dec_b_w", [48, 1], F32, isOutput=False)
    dec_a_bias = dp("dec_a_bias", [1, 1], F32, isOutput=False)
    dec_b_bias = dp("dec_b_bias", [1, 1], F32, isOutput=False)
    c1a_w = dp("c1a_w", [L, 48, 48], BF16, isOutput=False)   # [l][in][out]
    c2a_w = dp("c2a_w", [L, 48, 48], BF16, isOutput=False)
    c1b_w = dp("c1b_w", [L, 9, 48, 48], BF16, isOutput=False)
    c2b_w = dp("c2b_w", [L, 9, 48, 48], BF16, isOutput=False)
    c1a_b = dp("c1a_b", [L, 48, 1], F32, isOutput=False)
    c2a_b = dp("c2a_b", [L, 48, 1], F32, isOutput=False)
    c1b_b = dp("c1b_b", [L, 48, 1], F32, isOutput=False)
    c2b_b = dp("c2b_b", [L, 48, 1], F32, isOutput=False)
    ec_a = dp("ec_a", [WA, 64], BF16, isOutput=False)
    ec_b = dp("ec_b", [WB, 64], BF16, isOutput=False)
    erc_a = dp("erc_a", [64, 64], BF16, isOutput=False)
    erc_b = dp("erc_b", [64, 64], BF16, isOutput=False)
    grc_a = dp("grc_a", [64, 64], BF16, isOutput=False)
    grc_b = dp("grc_b", [64, 64], BF16, isOutput=False)
    gc_a = dp("gc_a", [64, WA], BF16, isOutput=False)
    gc_b = dp("gc_b", [64, WB], BF16, isOutput=False)
    ident = dp("ident", [128, 128], F32, isOutput=False)
    gmask = dp("gmask", [48, 2], F32, isOutput=False)   # [:,0]: row0 mask, [:,1]: row17 mask
    # A weights, transposed per core: [L, fl, n, ri, q, p]
    a_t = dp("a_t", [L, FL, M, 2, P, P], BF16, isOutput=False)

    oa = dp("oa", [1, RA, WA], F32, isOutput=True)
    ob = dp("ob", [1, RB, WB], F32, isOutput=True)
    if DBG:
        dbg_ub = dp("dbg_ub", [48, RBH, WB + 2], F32, isOutput=True)
        dbg_sb = dp("dbg_sb", [48, RB, WB], F32, isOutput=True)
        dbg_g1b = dp("dbg_g1b", [48, RBH - 2, WB + 2], F32, isOutput=True)
        dbg_ua = dp("dbg_ua", [48, RA, WA], F32, isOutput=True)

    rs_in = nc.dram_tensor("rs_in", [M, 2, P, M], F32)
    rs_out = nc.dram_tensor("rs_out", [FL, 2, P, M], F32)
    ag_in = nc.dram_tensor("ag_in", [FL, 2, P, M], BF16)
    ag_out = nc.dram_tensor("ag_out", [M, 2, P, M], BF16, addr_space="Shared")
    RG = [list(range(NC))]

    with TileContext(nc) as tc:
        with (
            tc.tile_pool(name="state", bufs=1) as st,
            tc.tile_pool(name="work", bufs=2) as wk,
            tc.tile_pool(name="aw", bufs=2) as awp,
            tc.tile_pool(name="ps", bufs=2, space="PSUM") as pp,
        ):
            # ---- resident tiles ----
            S_a = st.tile([48, RA, WA], F32, tag="S_a")
            S_b = st.tile([48, RB, WB], F32, tag="S_b")
            ST_a = st.tile([128, 2, 48, RA], BF16, tag="ST_a")   # [w%128, wh, ch, h]
            ST_b = st.tile([128, 48, 32], BF16, tag="ST_b")      # [w, ch, hpad32]
            ua_sp = st.tile([48, RA, WA], BF16, tag="ua_sp")
            ub_sp = st.tile([48, RBH, WB + 2], BF16, tag="ub_sp")
            g1a = st.tile([48, RA, WA], BF16, tag="g1a")
            g1b = st.tile([48, RBH - 2, WB + 2], BF16, tag="g1b")
            cEc_a = st.tile([128, 2, 64], BF16, tag="cEc_a")
            cEc_b = st.tile([128, 64], BF16, tag="cEc_b")
            cErc_a = st.tile([64, 64], BF16, tag="cErc_a")
            cErc_b = st.tile([64, 64], BF16, tag="cErc_b")
            cGrc_a = st.tile([64, 64], BF16, tag="cGrc_a")
            cGrc_b = st.tile([64, 64], BF16, tag="cGrc_b")
            cGc_a = st.tile([64, WA], BF16, tag="cGc_a")
            cGc_b = st.tile([64, WB], BF16, tag="cGc_b")
            cid = st.tile([128, 128], F32, tag="cid")
            w1a = st.tile([48, L, 48], BF16, tag="w1a")
            w2a = st.tile([48, L, 48], BF16, tag="w2a")
            w1b = st.tile([48, L, 9, 48], BF16, tag="w1b")
            w2b = st.tile([48, L, 9, 48], BF16, tag="w2b")
            b1a = st.tile([48, L], F32, tag="b1a")
            b2a = st.tile([48, L], F32, tag="b2a")
            b1b = st.tile([48, L], F32, tag="b1b")
            b2b = st.tile([48, L], F32, tag="b2b")
            benca = st.tile([48, 1], F32, tag="benca")
            bencb = st.tile([48, 1], F32, tag="bencb")
            wenca = st.tile([3, 48], BF16, tag="wenca")
            wencb = st.tile([3, 48], BF16, tag="wencb")
            wdeca = st.tile([48, 1], F32, tag="wdeca")
            wdecb = st.tile([48, 1], F32, tag="wdecb")
            bdeca = st.tile([1, 1], F32, tag="bdeca")
            bdecb = st.tile([1, 1], F32, tag="bdecb")
            gmk = st.tile([48, 2], F32, tag="gmk")
            xa16 = st.tile([3, RA, WA], BF16, tag="xa16")
            xb16 = st.tile([3, RB, WB], BF16, tag="xb16")

            dma = nc.sync.dma_start
            dmag = nc.gpsimd.dma_start

            for dst, src in (
                (cEc_a[:, 0, :], ec_a[0:128, :]), (cEc_a[:, 1, :], ec_a[128:256, :]),
                (cEc_b[:, :], ec_b[:, :]), (cErc_a[:, :], erc_a[:, :]),
                (cErc_b[:, :], erc_b[:, :]), (cGrc_a[:, :], grc_a[:, :]),
                (cGrc_b[:, :], grc_b[:, :]), (cGc_a[:, :], gc_a[:, :]),
                (cGc_b[:, :], gc_b[:, :]), (cid[:, :], ident[:, :]),
                (w1a[:, :, :], c1a_w.ap().rearrange("l i o -> i l o")),
                (w2a[:, :, :], c2a_w.ap().rearrange("l i o -> i l o")),
                (w1b[:, :, :, :], c1b_w.ap().rearrange("l t i o -> i l t o")),
                (w2b[:, :, :, :], c2b_w.ap().rearrange("l t i o -> i l t o")),
                (b1a[:, :], c1a_b.ap().rearrange("l i one -> i (l one)")),
                (b2a[:, :], c2a_b.ap().rearrange("l i one -> i (l one)")),
                (b1b[:, :], c1b_b.ap().rearrange("l i one -> i (l one)")),
                (b2b[:, :], c2b_b.ap().rearrange("l i one -> i (l one)")),
                (benca[:, :], enc_a_b[:, :]), (bencb[:, :], enc_b_b[:, :]),
                (wenca[:, :], enc_a_w[:, :]), (wencb[:, :], enc_b_w[:, :]),
                (wdeca[:, :], dec_a_w[:, :]), (wdecb[:, :], dec_b_w[:, :]),
                (bdeca[:, :], dec_a_bias[:, :]), (bdecb[:, :], dec_b_bias[:, :]),
                (gmk[:, :], gmask[:, :]),
            ):
                dma(out=dst, in_=src)

            nc.gpsimd.memset(ST_b[:, :, :], 0.0)
            nc.gpsimd.memset(ub_sp[:, :, :], 0.0)
            nc.gpsimd.memset(g1b[:, :, :], 0.0)

            # ---- encoders ----
            dmag(out=xa16[:, :, :], in_=ina[:, :, :])   # SWDGE cast f32->bf16
            dmag(out=xb16[:, :, :], in_=inb[:, :, :])
            xaf = xa16.rearrange("c r w -> c (r w)")
            xbf = xb16.rearrange("c r w -> c (r w)")
            Saf = S_a.rearrange("c r w -> c (r w)")
            Sbf = S_b.rearrange("c r w -> c (r w)")
            for j in range(16):
                pse = pp.tile([48, 512], F32, tag="psmall")
                nc.tensor.matmul(pse[:, :], wenca[:, :], xaf[:, j * 512:(j + 1) * 512],
                                 start=True, stop=True)
                nc.vector.tensor_scalar_add(Saf[:, j * 512:(j + 1) * 512], pse[:, :],
                                            benca[:, 0:1])
            for j in range(4):
                pse = pp.tile([48, 512], F32, tag="psmall")
                nc.tensor.matmul(pse[:, :], wencb[:, :], xbf[:, j * 512:(j + 1) * 512],
                                 start=True, stop=True)
                nc.vector.tensor_scalar_add(Sbf[:, j * 512:(j + 1) * 512], pse[:, :],
                                            bencb[:, 0:1])

            def build_ST():
                for wh in range(2):
                    for hg in range(4):
                        pst = pp.tile([128, 8, 48], F32, tag="psmall")
                        for hh in range(8):
                            h = hg * 8 + hh
                            nc.tensor.transpose(
                                pst[:, hh, :],
                                S_a[:, h, wh * 128:(wh + 1) * 128],
                                cid[0:48, 0:48])
                        nc.vector.tensor_copy(
                            ST_a[:, wh, :, hg * 8:hg * 8 + 8].rearrange("w c h -> w h c"),
                            pst[:, :, :])
                for hg in range(2):
                    pst = pp.tile([128, 8, 48], F32, tag="psmall")
                    for hh in range(8):
                        h = hg * 8 + hh
                        nc.tensor.transpose(pst[:, hh, :], S_b[:, h, :], cid[0:48, 0:48])
                    nc.vector.tensor_copy(
                        ST_b[:, :, hg * 8:hg * 8 + 8].rearrange("w c h -> w h c"),
                        pst[:, :, :])

            def fwd_dft(i):
                # stage 1 (contract w): z [(ri_c, n), (ch, h32)]
                psz_a = pp.tile([64, 3, 512], F32, tag="pbig")
                sta = ST_a.rearrange("w wh c h -> w (wh c h)")
                for c3 in range(3):
                    for wh in range(2):
                        nc.tensor.matmul(
                            psz_a[:, c3, :],
                            cEc_a[:, wh, :],
                            sta[:, wh * 1536 + c3 * 512: wh * 1536 + (c3 + 1) * 512],
                            start=(wh == 0), stop=(wh == 1))
                z_a = wk.tile([64, 1536], BF16, tag="z")
                nc.vector.tensor_copy(z_a[:, :], psz_a.rearrange("p a b -> p (a b)"))
                zt_a = wk.tile([64, 1536], BF16, tag="zt")
                nc.vector.transpose(zt_a[:, :], z_a[:, :])
                psz_b = pp.tile([64, 3, 512], F32, tag="pbig")
                stb = ST_b.rearrange("w c h -> w (c h)")
                for c3 in range(3):
                    nc.tensor.matmul(psz_b[:, c3, :], cEc_b[:, :],
                                     stb[:, c3 * 512:(c3 + 1) * 512],
                                     start=True, stop=True)
                z_b = wk.tile([64, 1536], BF16, tag="z")
                nc.vector.tensor_copy(z_b[:, :], psz_b.rearrange("p a b -> p (a b)"))
                zt_b = wk.tile([64, 1536], BF16, tag="zt")
                nc.vector.transpose(zt_b[:, :], z_b[:, :])
                # stage 2 (contract (ri_c, h)): coeff [(ri, f), (ch, n)]
                for name, zt, erc, qof in (("a", zt_a, cErc_a, 0), ("b", zt_b, cErc_b, 48)):
                    psc = pp.tile([64, 3, 512], F32, tag="pbig")
                    for c3 in range(3):
                        nc.tensor.matmul(psc[:, c3, :], erc[:, :],
                                         zt[:, c3 * 512:(c3 + 1) * 512],
                                         start=True, stop=True)
                    csb = wk.tile([64, 48, 32], F32, tag="csb")
                    nc.vector.tensor_copy(csb.rearrange("p c n -> p (c n)"),
                                          psc.rearrange("p a b -> p (a b)"))
                    dma(out=rs_in.ap().rearrange("f r q n -> r f q n")[:, :, qof:qof + 48, :],
                        in_=csb[:, :, :])
                nc.gpsimd.collective_compute(
                    "ReduceScatter", mybir.AluOpType.add, replica_groups=RG,
                    ins=[rs_in.ap().opt()], outs=[rs_out.ap().opt()])

            def mode_mix(i):
                cmix = wk.tile([96, FL, 2, M], BF16, tag="cmix")
                dmag(out=cmix[:, :, :, :],
                     in_=rs_out.ap().rearrange("f r q n -> q f r n"))
                cneg = wk.tile([96, FL, 2, M], BF16, tag="cneg")
                nc.vector.tensor_scalar_mul(cneg[:, :, 0, :], cmix[:, :, 1, :], -1.0)
                nc.vector.tensor_copy(cneg[:, :, 1, :], cmix[:, :, 0, :])
                psm = pp.tile([96, FL, M, 2], F32, tag="psmall")
                for c8 in range(8):
                    at = awp.tile([96, 16, 2, 96], BF16, tag="at")
                    dma(out=at[:, :, :, :],
                        in_=a_t.ap().rearrange("l f n r q p -> q l (f n) r p")[:, i, c8 * 16:(c8 + 1) * 16, :, :])
                    for m16 in range(16):
                        j = c8 * 16 + m16
                        fl, n = j // M, j % M
                        nc.tensor.matmul(psm[:, fl, n, :], at[:, m16, 0, :],
                                         cmix[:, fl, :, n], start=True, stop=False)
                        nc.tensor.matmul(psm[:, fl, n, :], at[:, m16, 1, :],
                                         cneg[:, fl, :, n], start=False, stop=True)
                oc = wk.tile([96, FL, 2, M], BF16, tag="oc")
                nc.vector.tensor_copy(oc[:, :, :, :],
                                      psm.rearrange("q f n r -> q f r n"))
                dma(out=ag_in.ap().rearrange("f r q n -> q f r n"), in_=oc[:, :, :, :])
                nc.gpsimd.collective_compute(
                    "AllGather", mybir.AluOpType.bypass, replica_groups=RG,
                    ins=[ag_in.ap().opt()], outs=[ag_out.ap().opt()])

            def inverse(i):
                cinv = wk.tile([64, 96, M], BF16, tag="cinv")
                dma(out=cinv[:, :, :],
                    in_=ag_out.ap().rearrange("f r q n -> r f q n"))
                civ = cinv.rearrange("p q n -> p (q n)")
                for name, grc, gc, nh, Wd, qof in (
                        ("a", cGrc_a, cGc_a, RA, WA, 0),
                        ("b", cGrc_b, cGc_b, RBH, WB, 48)):
                    pst = pp.tile([64, 3, 512], F32, tag="pbig")
                    for c3 in range(3):
                        nc.tensor.matmul(pst[:, c3, :], grc[:, :],
                                         civ[:, qof * M + c3 * 512: qof * M + (c3 + 1) * 512],
                                         start=True, stop=True)
                    t16 = wk.tile([64, 1536], BF16, tag="t16")
                    nc.vector.tensor_copy(t16[:, :], pst.rearrange("p a b -> p (a b)"))
                    tt = wk.tile([64, 48, 32], BF16, tag="tt")
                    nc.vector.transpose(tt.rearrange("p c h -> p (c h)"), t16[:, :])
                    per = 1024 // Wd                  # h rows per [48, per, Wd] psum tile
                    h = 0
                    while h < nh:
                        cnt = min(per, nh - h)
                        psy = pp.tile([48, per, Wd], F32, tag="pbig")
                        for hh in range(cnt):
                            nc.tensor.matmul(psy[:, hh, :], tt[:, :, h + hh],
                                             gc[:, :], start=True, stop=True)
                        if name == "a":
                            nc.vector.tensor_copy(
                                ua_sp[:, h:h + cnt, :],
                                psy[:, 0:cnt, :])
                        else:
                            nc.vector.tensor_copy(
                                ub_sp[:, h:h + cnt, 1:1 + WB],
                                psy[:, 0:cnt, :])
                        h += cnt

            def convs_a(i):
                uaf = ua_sp.rearrange("c r w -> c (r w)")
                g1f = g1a.rearrange("c r w -> c (r w)")
                for j in range(16):
                    ps1 = pp.tile([48, 512], F32, tag="psmall")
                    nc.tensor.matmul(ps1[:, :], w1a[:, i, :], uaf[:, j * 512:(j + 1) * 512],
                                     start=True, stop=True)
                    nc.scalar.activation(g1f[:, j * 512:(j + 1) * 512], ps1[:, :],
                                         GELU, bias=b1a[:, i:i + 1])
                for j in range(16):
                    ps2 = pp.tile([48, 512], F32, tag="psmall")
                    nc.tensor.matmul(ps2[:, :], w2a[:, i, :], g1f[:, j * 512:(j + 1) * 512],
                                     start=True, stop=True)
                    ut = wk.tile([48, 512], F32, tag="ut")
                    nc.scalar.activation(ut[:, :], ps2[:, :], GELU, bias=b2a[:, i:i + 1])
                    nc.vector.tensor_add(Saf[:, j * 512:(j + 1) * 512],
                                         Saf[:, j * 512:(j + 1) * 512], ut[:, :])

            def convs_b(i):
                # conv1: ub_sp rows r..r+2 -> g1b row r (r = 0..17)
                for j in range(5):
                    r0 = j * 4
                    cnt = min(4, 18 - r0)
                    psb = pp.tile([48, 4, WB], F32, tag="psmall")
                    for t in range(9):
                        dy, dx = t // 3, t % 3
                        nc.tensor.matmul(
                            psb[:, 0:cnt, :],
                            w1b[:, i, t, :],
                            ub_sp[:, r0 + dy:r0 + dy + cnt, dx:dx + WB],
                            start=(t == 0), stop=(t == 8))
                    nc.scalar.activation(
                        g1b[:, r0:r0 + cnt, 1:1 + WB],
                        psb[:, 0:cnt, :],
                        GELU, bias=b1b[:, i:i + 1])
                nc.vector.tensor_scalar_mul(g1b[:, 0, :], g1b[:, 0, :], gmk[:, 0:1])
                nc.vector.tensor_scalar_mul(g1b[:, 17, :], g1b[:, 17, :], gmk[:, 1:2])
                for j in range(4):
                    r0 = j * 4
                    psb = pp.tile([48, 4, WB], F32, tag="psmall")
                    for t in range(9):
                        dy, dx = t // 3, t % 3
                        nc.tensor.matmul(
                            psb[:, :, :],
                            w2b[:, i, t, :],
                            g1b[:, r0 + dy:r0 + dy + 4, dx:dx + WB],
                            start=(t == 0), stop=(t == 8))
                    ut = wk.tile([48, 4, WB], F32, tag="utb")
                    nc.scalar.activation(ut[:, :, :], psb[:, :, :],
                                         GELU, bias=b2b[:, i:i + 1])
                    nc.vector.tensor_add(
                        S_b[:, r0:r0 + 4, :], S_b[:, r0:r0 + 4, :], ut[:, :, :])

            for i in range(NL):
                build_ST()
                fwd_dft(i)
                mode_mix(i)
                inverse(i)
                convs_a(i)
                convs_b(i)
                if DBG and i == 0:
                    dmag(out=dbg_ub.ap(), in_=ub_sp[:, :, :])
                    dma(out=dbg_sb.ap(), in_=S_b[:, :, :])
                    dmag(out=dbg_g1b.ap(), in_=g1b[:, :, :])
                    dmag(out=dbg_ua.ap(), in_=ua_sp[:, :, :])

            # ---- decoders (fp32, one-time) ----
            oaf = oa.ap().rearrange("one r w -> one (r w)")
            obf = ob.ap().rearrange("one r w -> one (r w)")
            for c6 in range(6):            # 6 chunks of 1536 - but 8192 = 5x1536 + 512
                w0 = c6 * 1536
                wid = min(1536, RA * WA - w0)
                nchk = (wid + 511) // 512
                psd = pp.tile([1, 3, 512], F32, tag="pbig")
                for j in range(nchk):
                    nc.tensor.matmul(psd[:, j, :], wdeca[:, :],
                                     Saf[:, w0 + j * 512: w0 + (j + 1) * 512],
                                     start=True, stop=True)
                od = wk.tile([1, 1536], F32, tag="od")
                nc.vector.tensor_scalar_add(od[:, 0:wid],
                                            psd.rearrange("p a b -> p (a b)")[:, 0:wid],
                                            bdeca[:, 0:1])
                dma(out=oaf[:, w0:w0 + wid], in_=od[:, 0:wid])
            for c6 in range(2):
                w0 = c6 * 1024
                psd = pp.tile([1, 3, 512], F32, tag="pbig")
                for j in range(2):
                    nc.tensor.matmul(psd[:, j, :], wdecb[:, :],
                                     Sbf[:, w0 + j * 512: w0 + (j + 1) * 512],
                                     start=True, stop=True)
                od = wk.tile([1, 1536], F32, tag="od")
                nc.vector.tensor_scalar_add(od[:, 0:1024],
                                            psd.rearrange("p a b -> p (a b)")[:, 0:1024],
                                            bdecb[:, 0:1])
                dma(out=obf[:, w0:w0 + 1024], in_=od[:, 0:1024])

    nc.finalize()
    return nc


def _prep(inputs):
    """Host-side: build the 8 per-core input maps."""
    A_re, A_im = np.asarray(inputs["A_re"]), np.asarray(inputs["A_im"])
    shared = {}
    shared["enc_a_w"] = _bf(np.asarray(inputs["enc_a_w"]).T)
    shared["enc_b_w"] = _bf(np.asarray(inputs["enc_b_w"]).T)
    shared["enc_a_b"] = _f32(np.asarray(inputs["enc_a_b"])[:, None])
    shared["enc_b_b"] = _f32(np.asarray(inputs["enc_b_b"])[:, None])
    shared["dec_a_w"] = _f32(np.asarray(inputs["dec_a_w"]).T)
    shared["dec_b_w"] = _f32(np.asarray(inputs["dec_b_w"]).T)
    shared["dec_a_bias"] = _f32(np.asarray(inputs["dec_a_b"]).reshape(1, 1))
    shared["dec_b_bias"] = _f32(np.asarray(inputs["dec_b_b"]).reshape(1, 1))
    shared["c1a_w"] = _bf(np.asarray(inputs["c1a_w"]).transpose(0, 2, 1))
    shared["c2a_w"] = _bf(np.asarray(inputs["c2a_w"]).transpose(0, 2, 1))
    shared["c1b_w"] = _bf(
        np.asarray(inputs["c1b_w"]).transpose(0, 3, 4, 2, 1).reshape(L, 9, 48, 48))
    shared["c2b_w"] = _bf(
        np.asarray(inputs["c2b_w"]).transpose(0, 3, 4, 2, 1).reshape(L, 9, 48, 48))
    shared["c1a_b"] = _f32(np.asarray(inputs["c1a_b"])[..., None])
    shared["c2a_b"] = _f32(np.asarray(inputs["c2a_b"])[..., None])
    shared["c1b_b"] = _f32(np.asarray(inputs["c1b_b"])[..., None])
    shared["c2b_b"] = _f32(np.asarray(inputs["c2b_b"])[..., None])
    shared["ident"] = np.eye(128, dtype=np.float32)

    ia = _f32(np.concatenate([np.asarray(inputs["x_a"]), np.asarray(inputs["u_a"])], 0))
    ib = _f32(np.concatenate([np.asarray(inputs["x_b"]), np.asarray(inputs["u_b"])], 0))

    maps = []
    for k in range(NC):
        m = dict(shared)
        m.update(_dft_consts(k))
        gm = np.ones((48, 2), np.float32)
        if k == 0:
            gm[:, 0] = 0.0
        if k == NC - 1:
            gm[:, 1] = 0.0
        m["gmask"] = gm
        m["ina"] = np.ascontiguousarray(ia[:, k * RA:(k + 1) * RA, :])
        m["inb"] = np.ascontiguousarray(ib[:, k * RB:(k + 1) * RB, :])
        # a_t[l, fl, n, ri, q, p] = A_{ri}[l, p, q, 4k+fl, n]
        blk_re = A_re[:, :, :, 4 * k:4 * k + 4, :]   # [L, p, q, fl, n]
        blk_im = A_im[:, :, :, 4 * k:4 * k + 4, :]
        at = np.stack([blk_re, blk_im], axis=3)      # [L, p, q, ri, fl, n]
        m["a_t"] = _bf(at.transpose(0, 4, 5, 3, 2, 1))
        maps.append(m)
    return maps


def kernel(**inputs):
    if "nc" not in _CACHE:
        _CACHE["nc"] = _build()
    maps = _prep(inputs)
    res = run_bass_kernel_spmd(_CACHE["nc"], maps, core_ids=list(range(NC)))
    oa = np.concatenate([res.results[k]["oa"] for k in range(NC)], axis=1)
    ob = np.concatenate([res.results[k]["ob"] for k in range(NC)], axis=1)
    return (oa.astype(np.float32), ob.astype(np.float32))
